# revision 38
# baseline (speedup 1.0000x reference)
"""Trainium2 Bass kernel for a dense transformer block (pre-LN, 16-head causal
attention + 3x FFN), distributed over 8 NeuronCores.

v3 design
---------
Sharding: tensor-parallel over heads (2 heads/core, both batch elements on
every core) for QKV/attention; two 8-core AllToAlls (one per batch element)
redistribute the per-head attention context to token-parallel shards
(256 tokens of each batch per core) for the output projection, LN2 and FFN.

 - LayerNorm 1 applied on the host; QKV is a plain matmul + per-channel bias.
 - Softmax normalization deferred: raw ctx + Z rows ship through the
   AllToAll; phase C normalizes via K=2 broadcast matmul + reciprocal.
 - One exp() per 128-token score tile covers both heads (2-bank PSUM tile).
 - All host-side arrays are partition-major so every DMA moves multi-KB
   contiguous lines per partition.
 - Phase C runs in two column halves (one per batch): half 0 only needs
   AllToAll#0, so it starts while AllToAll#1 is still in flight, and each
   half's LN2 scalar chain hides under the other half's FFN matmuls.
"""

import numpy as np
import ml_dtypes

B, T, C = 2, 2048, 1024
NH, H = 16, 64
FF = 3 * C
EPS = 1e-6
N_CORES = 8
TT = B * T            # 4096 tokens (head-parallel phase works on all)
TS = TT // N_CORES    # 512 tokens per core in phase C (256 from each batch)
TQ = TS // B          # 256 tokens per (batch, core)
HPC = NH // N_CORES   # 2 heads per core
HD2 = HPC * H         # 128

BF16 = ml_dtypes.bfloat16

_BUILT = {}

NT = TT // 128        # 32 token tiles
NKC = C // 128        # 8 channel k-tiles
NMF = FF // 128       # 24 ff tiles


def _build():
    import concourse.bacc as bacc
    import concourse.mybir as mybir
    import concourse.tile as tile
    dt = mybir.dt
    alu = mybir.AluOpType
    act = mybir.ActivationFunctionType

    nc = bacc.Bacc("TRN2", target_bir_lowering=False, debug=False,
                   num_devices=N_CORES)

    # ----- kernel I/O (per-core shards; all partition-major) -----
    p_xn = nc.declare_dram_parameter("p_xn", [128, TT // 512, NKC, 512], dt.bfloat16, isOutput=False)
    p_wq = nc.declare_dram_parameter("p_wq", [128, NKC, HD2], dt.bfloat16, isOutput=False)
    p_wk = nc.declare_dram_parameter("p_wk", [128, NKC, HD2], dt.bfloat16, isOutput=False)
    p_wv = nc.declare_dram_parameter("p_wv", [128, NKC, HD2], dt.bfloat16, isOutput=False)
    p_bqkv = nc.declare_dram_parameter("p_bqkv", [HD2, 3], dt.float32, isOutput=False)
    p_wo = nc.declare_dram_parameter("p_wo", [128, NKC, NKC, 128], dt.bfloat16, isOutput=False)
    p_w1 = nc.declare_dram_parameter("p_w1", [128, NMF, NKC, 128], dt.bfloat16, isOutput=False)
    p_b1c = nc.declare_dram_parameter("p_b1c", [128, NMF], dt.float32, isOutput=False)
    p_w2 = nc.declare_dram_parameter("p_w2", [128, NKC, NMF, 128], dt.bfloat16, isOutput=False)
    p_b2c = nc.declare_dram_parameter("p_b2c", [128, NKC], dt.float32, isOutput=False)
    p_xts = nc.declare_dram_parameter("p_xts", [128, NKC, TS], dt.bfloat16, isOutput=False)
    p_ind2 = nc.declare_dram_parameter("p_ind2", [2, 128], dt.bfloat16, isOutput=False)
    p_maskd = nc.declare_dram_parameter("p_maskd", [128, 128], dt.bfloat16, isOutput=False)
    p_ident = nc.declare_dram_parameter("p_ident", [128, 128], dt.bfloat16, isOutput=False)
    p_out = nc.declare_dram_parameter("p_out", [C, TS], dt.float32, isOutput=True)

    with tile.TileContext(nc, num_cores=N_CORES) as tc:
        with (
            tc.tile_pool(name="persist", bufs=1) as pp,
            tc.tile_pool(name="dram", bufs=1, space="DRAM") as pdram,
        ):
            # ------------- constants, phase-A-critical DMAs first -------------
            wq = pp.tile([128, NKC, HD2], dt.bfloat16)
            nc.sync.dma_start(wq[:], p_wq[:])
            bqkv = pp.tile([HD2, 3], dt.float32)
            nc.sync.dma_start(bqkv[:], p_bqkv[:])
            ident = pp.tile([128, 128], dt.bfloat16)
            nc.sync.dma_start(ident[:], p_ident[:])
            wk = pp.tile([128, NKC, HD2], dt.bfloat16)
            wv = pp.tile([128, NKC, HD2], dt.bfloat16)
            maskd = pp.tile([128, 128], dt.bfloat16)
            ones128_row = pp.tile([1, 128], dt.bfloat16)
            nc.vector.memset(ones128_row[:], 1.0)
            isc_col = pp.tile([128, 1], dt.bfloat16)   # 1/1024 column for LN2 sums
            nc.vector.memset(isc_col[:], 1.0 / C)
            ind2 = pp.tile([2, 128], dt.bfloat16)      # Z broadcast selector
            b1c = pp.tile([128, NMF], dt.float32)
            b2c = pp.tile([128, NKC], dt.float32)

            # phase C prefetched weights / residual (persist through the run)
            wo_all = pp.tile([128, NKC, NKC, 128], dt.bfloat16)
            w1_all = pp.tile([128, NMF, NKC, 128], dt.bfloat16)
            w2_all = pp.tile([128, NKC, NMF, 128], dt.bfloat16)
            xts = pp.tile([128, NKC, TS], dt.bfloat16)

            # collective staging (DRAM)
            cc_in = [pdram.tile([N_CORES, 130, TQ], dt.bfloat16, name=f"ccin{b}")
                     for b in range(B)]
            cc_out = [pdram.tile([N_CORES, 130, TQ], dt.bfloat16, name=f"ccout{b}")
                      for b in range(B)]

            with tc.tile_pool(name="abact", bufs=1) as pab:
                # activation tensors that live through phases A+B only
                qT = pab.tile([128, TT], dt.bfloat16)
                kT = pab.tile([128, TT], dt.bfloat16)
                v = pab.tile([128, NT, 2, 65], dt.bfloat16)
                ctxT = pab.tile([128, TT], dt.bfloat16)
                # softmax denominators, one single-row tile per head (writes
                # must start at partition 0)
                zrow = [pab.tile([1, TT], dt.bfloat16, name=f"zrow{h}")
                        for h in range(2)]

                # ---------------- stage A: QKV ----------------
                with (
                    tc.tile_pool(name="xin", bufs=3) as pxt,
                    tc.tile_pool(name="vtev", bufs=2) as pvte,
                    tc.tile_pool(name="apsum", bufs=3, space="PSUM") as pps_a,
                    tc.tile_pool(name="apsum1", bufs=2, space="PSUM") as pps_a1,
                ):
                    nc.vector.memset(v[:, :, :, 64], 1.0)
                    xnt0 = pxt.tile([128, NKC, 512], dt.bfloat16, tag="xt")
                    nc.sync.dma_start(xnt0[:], p_xn[:, 0, :, :])
                    # remaining params queue behind the first activation chunk
                    nc.sync.dma_start(wk[:], p_wk[:])
                    nc.sync.dma_start(wv[:], p_wv[:])
                    nc.sync.dma_start(maskd[:], p_maskd[:])
                    nc.sync.dma_start(ind2[:], p_ind2[:])
                    nc.sync.dma_start(b1c[:], p_b1c[:])
                    nc.sync.dma_start(b2c[:], p_b2c[:])
                    for ch in range(TT // 512):
                        sl = slice(512 * ch, 512 * (ch + 1))
                        if ch == 0:
                            xnt = xnt0
                        else:
                            xnt = pxt.tile([128, NKC, 512], dt.bfloat16, tag="xt")
                            nc.sync.dma_start(xnt[:], p_xn[:, ch, :, :])
                        vT = pvte.tile([128, 512], dt.bfloat16, tag="vt")
                        for idx, (w, dst) in enumerate(
                                ((wq, qT), (wk, kT), (wv, None))):
                            ps = pps_a.tile([128, 512], dt.float32, tag="qkv")
                            for k in range(NKC):
                                nc.tensor.matmul(ps[:], w[:, k, :], xnt[:, k, :],
                                                 start=(k == 0), stop=(k == NKC - 1))
                            if idx == 0:
                                nc.scalar.activation(dst[:, sl], ps[:], act.Identity,
                                                     bias=bqkv[:, idx:idx + 1])
                            elif idx == 1:
                                nc.vector.tensor_scalar(dst[:, sl], ps[:],
                                                        bqkv[:, idx:idx + 1], None,
                                                        alu.add)
                            else:
                                nc.vector.tensor_scalar(vT[:], ps[:],
                                                        bqkv[:, idx:idx + 1], None,
                                                        alu.add)
                        # v_aug [s, tile, head, 65] via PE transpose of vT
                        for i in range(4):
                            pvt = pps_a1.tile([128, 128], dt.bfloat16, tag="vtp")
                            nc.tensor.transpose(pvt[:], vT[:, 128 * i:128 * (i + 1)],
                                                ident[:])
                            nc.scalar.copy(v[:, 4 * ch + i, :, 0:64],
                                           pvt[:].rearrange("p (h d) -> p h d", h=2))
                        # interleave phase-C prefetch pieces so the DMA queue
                        # stays just ahead of compute without starving the
                        # critical xn chunk loads
                        nc.sync.dma_start(xts[:, ch, :], p_xts[:, ch, :])
                        nc.sync.dma_start(wo_all[:, ch, :, :], p_wo[:, ch, :, :])
                        for mf in (2 * ch, 2 * ch + 1):
                            nc.sync.dma_start(w1_all[:, mf, :, :],
                                              p_w1[:, mf, :, :])

                    # remaining prefetch (finishes early in stage B, ahead of
                    # the first AllToAll's staging DMAs)
                    for mf in range(16, NMF):
                        nc.sync.dma_start(w1_all[:, mf, :, :], p_w1[:, mf, :, :])
                    nc.sync.dma_start(w2_all[:], p_w2[:])

                # ---------------- stage B: attention ----------------
                with (
                    tc.tile_pool(name="exps", bufs=4) as pexp,
                    tc.tile_pool(name="scpsum", bufs=2, space="PSUM") as pps_sc,
                    tc.tile_pool(name="ctxpsum", bufs=2, space="PSUM") as pps_ctx,
                ):
                    for b in range(B):
                        for qt in range(T // 512):
                            G = b * T + 512 * qt
                            gsl = slice(G, G + 512)
                            nj = 4 * qt + 4
                            pcs = pps_ctx.tile([65, 2, 512], dt.float32, tag="ctx")
                            ets = []
                            for j in range(nj):
                                st = b * (T // 128) + j   # global s-tile index
                                sp = pps_sc.tile([128, 2, 512], dt.float32, tag="sc")
                                for h in range(2):
                                    hsl = slice(64 * h, 64 * (h + 1))
                                    nc.tensor.matmul(
                                        sp[:, h, :],
                                        kT[hsl, 128 * st:128 * (st + 1)],
                                        qT[hsl, gsl], start=True, stop=True)
                                et = pexp.tile([128, 2, 512], dt.bfloat16, tag="et")
                                if j >= nj - 4:
                                    off = j - (nj - 4)
                                    if off > 0:
                                        nc.gpsimd.memset(et[:, :, 0:128 * off], 0.0)
                                    for h in range(2):
                                        nc.scalar.activation(
                                            et[:, h, 128 * off:512],
                                            sp[:, h, 128 * off:512],
                                            act.Exp,
                                            scale=1.0 / float(np.sqrt(H)))
                                        nc.gpsimd.tensor_tensor(
                                            et[:, h, 128 * off:128 * (off + 1)],
                                            et[:, h, 128 * off:128 * (off + 1)],
                                            maskd[:], alu.mult)
                                else:
                                    # one single-bank exp per head (faster
                                    # than one activation spanning 2 banks)
                                    for h in range(2):
                                        nc.scalar.activation(
                                            et[:, h, :], sp[:, h, :],
                                            act.Exp,
                                            scale=1.0 / float(np.sqrt(H)))
                                ets.append(et)
                                # software pipeline: AV for tile j-1 after scores j
                                if j > 0:
                                    for h in range(2):
                                        nc.tensor.matmul(
                                            pcs[:, h, :],
                                            v[:, b * (T // 128) + j - 1, h, :],
                                            ets[j - 1][:, h, :],
                                            start=(j - 1 == 0), stop=False)
                            for h in range(2):
                                nc.tensor.matmul(
                                    pcs[:, h, :], v[:, b * (T // 128) + nj - 1, h, :],
                                    ets[nj - 1][:, h, :],
                                    start=(nj == 1), stop=True)
                            # evict raw ctx + Z (normalization deferred to stage C)
                            for h in range(2):
                                nc.vector.tensor_copy(ctxT[64 * h:64 * (h + 1), gsl],
                                                      pcs[0:64, h, :])
                                nc.vector.tensor_copy(zrow[h][:, gsl],
                                                      pcs[64:65, h, :])
                            # this 512-token chunk feeds dst cores 2qt, 2qt+1;
                            # stage via the gpsimd DMA queue so the AllToAll
                            # never waits behind weight prefetch on sync
                            for j2 in (2 * qt, 2 * qt + 1):
                                tsl = slice(b * T + TQ * j2, b * T + TQ * (j2 + 1))
                                nc.sync.dma_start(cc_in[b][j2, 0:128, :],
                                                  ctxT[:, tsl])
                                for h in range(2):
                                    nc.sync.dma_start(cc_in[b][j2, 128 + h, :],
                                                      zrow[h][:, tsl])
                        nc.gpsimd.collective_compute(
                            "AllToAll", alu.bypass,
                            replica_groups=[list(range(N_CORES))],
                            ins=[cc_in[b].opt()],
                            outs=[cc_out[b].opt()],
                        )

            # ---------------- stage C: Wo + LN2 + FFN ----------------
            # processed in two column halves (one per batch element) so half 0
            # starts as soon as AllToAll#0 lands and half 1's weights/stats
            # chain hides under half 0's FFN matmuls
            with (
                tc.tile_pool(name="postsb", bufs=1) as pq,
                tc.tile_pool(name="evict", bufs=3) as pev,
                tc.tile_pool(name="ln2tmp", bufs=1) as pl2,
                tc.tile_pool(name="ffpsum", bufs=3, space="PSUM") as pps_ff,
                tc.tile_pool(name="npsum", bufs=2, space="PSUM") as pps_n,
                tc.tile_pool(name="cpsum", bufs=1, space="PSUM") as pps_c,
            ):
                ctxC = pq.tile([128, NKC, TS], dt.bfloat16)
                zF = pq.tile([2, NKC, TS], dt.bfloat16)
                r2b = pq.tile([128, NKC, TS], dt.bfloat16)
                xn2T = pq.tile([128, NKC, TS], dt.bfloat16)
                hT = pq.tile([128, NMF, TS], dt.bfloat16)

                for half in range(B):
                    csl = slice(TQ * half, TQ * (half + 1))
                    # input DMAs ride the scalar queue: the sync queue is
                    # still draining the second batch's AllToAll staging
                    for j2 in range(N_CORES):
                        nc.scalar.dma_start(zF[:, j2, csl],
                                            cc_out[half][j2, 128:130, :])
                        nc.scalar.dma_start(ctxC[:, j2, csl],
                                            cc_out[half][j2, 0:128, :])

                    # normalize ctx by 1/Z in place: broadcast Z via K=2
                    # matmul, reciprocal on the broadcast tile, multiply
                    for k in range(NKC):
                        pz = pps_n.tile([128, TQ], dt.float32, tag="nz")
                        nc.tensor.matmul(pz[:], ind2[:], zF[:, k, csl],
                                         start=True, stop=True)
                        zinvb = pev.tile([128, TQ], dt.float32, tag="zi")
                        nc.vector.reciprocal_approx_fast(zinvb[:], pz[:])
                        nc.vector.tensor_tensor(ctxC[:, k, csl], ctxC[:, k, csl],
                                                zinvb[:], alu.mult)

                    # Wo + residual (residual kept in bf16)
                    for mc in range(NKC):
                        ps = pps_ff.tile([128, TQ], dt.float32, tag="ff")
                        for k in range(NKC):
                            nc.tensor.matmul(ps[:], wo_all[:, mc, k, :],
                                             ctxC[:, k, csl],
                                             start=(k == 0), stop=(k == NKC - 1))
                        nc.vector.tensor_tensor(r2b[:, mc, csl], ps[:],
                                                xts[:, mc, csl], alu.add)

                    # LN2 partition sums (mean, mean of square)
                    ps1 = pps_c.tile([1, TQ], dt.float32, tag="s1")
                    ps2 = pps_c.tile([1, TQ], dt.float32, tag="s2")
                    for mc in range(NKC):
                        sqt = pev.tile([128, TQ], dt.bfloat16, tag="sq")
                        nc.gpsimd.tensor_tensor(sqt[:], r2b[:, mc, csl],
                                                r2b[:, mc, csl], alu.mult)
                        nc.tensor.matmul(ps1[:], isc_col[:], r2b[:, mc, csl],
                                         start=(mc == 0), stop=(mc == NKC - 1))
                        nc.tensor.matmul(ps2[:], isc_col[:], sqt[:],
                                         start=(mc == 0), stop=(mc == NKC - 1))
                    muf = pl2.tile([1, TQ], dt.float32, tag="muf")
                    nc.vector.tensor_copy(muf[:], ps1[:])
                    varf = pl2.tile([1, TQ], dt.float32, tag="varf")
                    nc.vector.tensor_tensor(varf[:], muf[:], muf[:], alu.mult)
                    nc.vector.tensor_tensor(varf[:], ps2[:], varf[:], alu.subtract)
                    sdr = pl2.tile([1, TQ], dt.float32, tag="sdr")
                    nc.scalar.activation(sdr[:], varf[:], act.Sqrt,
                                         scale=float(C) / (C - 1))
                    mu2row = pl2.tile([1, TQ], dt.bfloat16, tag="mu2")
                    nc.vector.tensor_copy(mu2row[:], muf[:])
                    sd2row = pl2.tile([1, TQ], dt.bfloat16, tag="sd2")
                    nc.vector.tensor_copy(sd2row[:], sdr[:])
                    pmb = pps_c.tile([128, TQ], dt.float32, tag="bcast")
                    nc.tensor.matmul(pmb[:], ones128_row[:], mu2row[:],
                                     start=True, stop=True)
                    m2b = pl2.tile([128, TQ], dt.bfloat16, tag="m2b")
                    nc.scalar.copy(m2b[:], pmb[:])
                    pib = pps_c.tile([128, TQ], dt.float32, tag="bcast")
                    nc.tensor.matmul(pib[:], ones128_row[:], sd2row[:],
                                     start=True, stop=True)
                    i2b = pl2.tile([128, TQ], dt.float32, tag="i2b")
                    nc.vector.reciprocal_approx_fast(i2b[:], pib[:])

                    for mc in range(NKC):
                        tmp = pev.tile([128, TQ], dt.bfloat16, tag="xtmp")
                        nc.gpsimd.tensor_tensor(tmp[:], r2b[:, mc, csl], m2b[:],
                                                alu.subtract)
                        nc.vector.tensor_tensor(xn2T[:, mc, csl], tmp[:], i2b[:],
                                                alu.mult)

                    # ---- FFN ----
                    for mf in range(NMF):
                        ps = pps_ff.tile([128, TQ], dt.float32, tag="ff")
                        for k in range(NKC):
                            nc.tensor.matmul(ps[:], w1_all[:, mf, k, :],
                                             xn2T[:, k, csl],
                                             start=(k == 0), stop=(k == NKC - 1))
                        nc.scalar.activation(hT[:, mf, csl], ps[:], act.Relu,
                                             bias=b1c[:, mf:mf + 1])

                    for mc in range(NKC):
                        ps = pps_ff.tile([128, TQ], dt.float32, tag="ff")
                        for k in range(NMF):
                            nc.tensor.matmul(ps[:], w2_all[:, mc, k, :],
                                             hT[:, k, csl],
                                             start=(k == 0), stop=(k == NMF - 1))
                        ot = pev.tile([128, TQ], dt.float32, tag="ot")
                        nc.vector.scalar_tensor_tensor(ot[:], ps[:],
                                                       b2c[:, mc:mc + 1],
                                                       r2b[:, mc, csl],
                                                       alu.add, alu.add)
                        nc.sync.dma_start(p_out[128 * mc:128 * (mc + 1), csl],
                                          ot[:])

    nc.compile()
    return nc


def _host_prep(inputs):
    """Fold LN affines into weights, apply LN1 on host, build per-core maps.

    All device-visible arrays are laid out partition-major ([128, ...]) so
    DMAs move long contiguous lines per partition.
    """
    x = np.asarray(inputs["x"], np.float32)
    Wq = np.asarray(inputs["Wq"], np.float32)
    Wk = np.asarray(inputs["Wk"], np.float32)
    Wv = np.asarray(inputs["Wv"], np.float32)
    Wo = np.asarray(inputs["Wo"], np.float32)
    bo = np.asarray(inputs["bo"], np.float32)
    W1 = np.asarray(inputs["W1"], np.float32)
    b1 = np.asarray(inputs["b1"], np.float32)
    W2 = np.asarray(inputs["W2"], np.float32)
    b2 = np.asarray(inputs["b2"], np.float32)
    g1 = np.asarray(inputs["g1"], np.float32)
    be1 = np.asarray(inputs["be1"], np.float32)
    g2 = np.asarray(inputs["g2"], np.float32)
    be2 = np.asarray(inputs["be2"], np.float32)

    xf = x.reshape(TT, C)                      # both batches stacked
    # LN1 on host (elementwise prep; torch: unbiased std, eps added to std)
    mu = xf.mean(axis=1, keepdims=True)
    sd = np.sqrt(xf.var(axis=1, ddof=1, keepdims=True)) + EPS
    xn = (xf - mu) / sd                        # gamma folded into Wq/Wk/Wv
    # [C, TT] -> partition-major [128, n_chunk, NKC, 512]
    xnP = np.ascontiguousarray(
        xn.T.reshape(NKC, 128, TT // 512, 512).transpose(1, 2, 0, 3))

    def fold_qkv(W):
        Weff = g1[:, None] * W                  # [NH, C, H] with g1 on C
        Weff = np.ascontiguousarray(np.transpose(Weff, (1, 0, 2)))  # [C, NH, H]
        bias = np.einsum("c,hck->hk", be1, W)   # [NH, H]
        return Weff, bias

    Wq_e, bq = fold_qkv(Wq)
    Wk_e, bk = fold_qkv(Wk)
    Wv_e, bv = fold_qkv(Wv)

    woT = np.ascontiguousarray(Wo.T)            # [NH*H, C]
    w1T = np.ascontiguousarray(g2[:, None] * W1.T)   # [C, FF]
    b1_eff = b1 + be2 @ W1.T                         # [FF]
    w2T = np.ascontiguousarray(W2.T)            # [FF, C]

    # partition-major blocked weights
    # wo: [C, C] -> [p, mc, k, 128] with row k*128+p of block mc
    woP = np.ascontiguousarray(
        woT.reshape(NKC, 128, NKC, 128).transpose(1, 2, 0, 3))
    w1P = np.ascontiguousarray(
        w1T.reshape(NKC, 128, NMF, 128).transpose(1, 2, 0, 3))
    w2P = np.ascontiguousarray(
        w2T.reshape(NMF, 128, NKC, 128).transpose(1, 2, 0, 3))

    tq = np.arange(128)[None, :]
    s = np.arange(128)[:, None]
    maskd = (s <= tq).astype(BF16)

    shared = {
        "p_xn": xnP.astype(BF16),
        "p_wo": woP.astype(BF16),
        "p_w1": w1P.astype(BF16),
        "p_b1c": np.ascontiguousarray(
            b1_eff.reshape(NMF, 128).T).astype(np.float32),
        "p_w2": w2P.astype(BF16),
        "p_b2c": np.ascontiguousarray(
            b2.reshape(NKC, 128).T).astype(np.float32),
        "p_ind2": np.repeat(np.eye(2, dtype=np.float32), 64, axis=1).astype(BF16),
        "p_maskd": maskd,
        "p_ident": np.eye(128, dtype=np.float32).astype(BF16),
    }

    in_maps = []
    for r in range(N_CORES):
        h0 = HPC * r
        hs = slice(h0, h0 + HPC)
        m = dict(shared)
        for nm, We in (("p_wq", Wq_e), ("p_wk", Wk_e), ("p_wv", Wv_e)):
            wr = We[:, hs, :].reshape(C, HD2)        # [C, 128]
            m[nm] = np.ascontiguousarray(
                wr.reshape(NKC, 128, HD2).transpose(1, 0, 2)).astype(BF16)
        m["p_bqkv"] = np.ascontiguousarray(
            np.stack([bq[hs].reshape(HD2), bk[hs].reshape(HD2),
                      bv[hs].reshape(HD2)], axis=1)).astype(np.float32)
        # residual stream for this core's tokens: 256 from each batch,
        # with the Wo bias folded in; partition-major [128, NKC, TS]
        xts = np.concatenate(
            [x[b, TQ * r:TQ * (r + 1), :].T for b in range(B)], axis=1)
        xts = xts + bo[:, None]                      # [C, TS]
        m["p_xts"] = np.ascontiguousarray(
            xts.reshape(NKC, 128, TS).transpose(1, 0, 2)).astype(BF16)
        in_maps.append(m)
    return in_maps


def kernel(**inputs) -> np.ndarray:
    from concourse.bass_utils import run_bass_kernel_spmd

    if "nc" not in _BUILT:
        _BUILT["nc"] = _build()
    nc = _BUILT["nc"]

    in_maps = _host_prep(inputs)
    res = run_bass_kernel_spmd(nc, in_maps, core_ids=list(range(N_CORES)))

    out = np.empty((B, T, C), np.float32)
    for r in range(N_CORES):
        po = res.results[r]["p_out"]
        for b in range(B):
            out[b, TQ * r:TQ * (r + 1), :] = po[:, TQ * b:TQ * (b + 1)].T
    return out


# revision 44
# speedup vs baseline: 1.0391x; 1.0391x over previous
"""Trainium2 Bass kernel for a dense transformer block (pre-LN, 16-head causal
attention + 3x FFN), distributed over 8 NeuronCores.

v3 design
---------
Sharding: tensor-parallel over heads (2 heads/core, both batch elements on
every core) for QKV/attention; two 8-core AllToAlls (one per batch element)
redistribute the per-head attention context to token-parallel shards
(256 tokens of each batch per core) for the output projection, LN2 and FFN.

 - LayerNorm 1 applied on the host; QKV is a plain matmul + per-channel bias.
 - Softmax normalization deferred: raw ctx + Z rows ship through the
   AllToAll; phase C normalizes via K=2 broadcast matmul + reciprocal.
 - One exp() per 128-token score tile covers both heads (2-bank PSUM tile).
 - All host-side arrays are partition-major so every DMA moves multi-KB
   contiguous lines per partition.
 - Phase C runs in two column halves (one per batch): half 0 only needs
   AllToAll#0, so it starts while AllToAll#1 is still in flight, and each
   half's LN2 scalar chain hides under the other half's FFN matmuls.
"""

import numpy as np
import ml_dtypes

B, T, C = 2, 2048, 1024
NH, H = 16, 64
FF = 3 * C
EPS = 1e-6
N_CORES = 8
TT = B * T            # 4096 tokens (head-parallel phase works on all)
TS = TT // N_CORES    # 512 tokens per core in phase C (256 from each batch)
TQ = TS // B          # 256 tokens per (batch, core)
HPC = NH // N_CORES   # 2 heads per core
HD2 = HPC * H         # 128

BF16 = ml_dtypes.bfloat16

_BUILT = {}

NT = TT // 128        # 32 token tiles
NKC = C // 128        # 8 channel k-tiles
NMF = FF // 128       # 24 ff tiles


def _build():
    import concourse.bacc as bacc
    import concourse.mybir as mybir
    import concourse.tile as tile
    dt = mybir.dt
    alu = mybir.AluOpType
    act = mybir.ActivationFunctionType

    nc = bacc.Bacc("TRN2", target_bir_lowering=False, debug=False,
                   num_devices=N_CORES)

    # ----- kernel I/O (per-core shards; all partition-major) -----
    p_xn = nc.declare_dram_parameter("p_xn", [128, TT // 512, NKC, 512], dt.bfloat16, isOutput=False)
    p_wq = nc.declare_dram_parameter("p_wq", [128, NKC, HD2], dt.bfloat16, isOutput=False)
    p_wk = nc.declare_dram_parameter("p_wk", [128, NKC, HD2], dt.bfloat16, isOutput=False)
    p_wv = nc.declare_dram_parameter("p_wv", [128, NKC, HD2], dt.bfloat16, isOutput=False)
    p_bqkv = nc.declare_dram_parameter("p_bqkv", [HD2, 3], dt.float32, isOutput=False)
    p_wo = nc.declare_dram_parameter("p_wo", [128, NKC, NKC, 128], dt.bfloat16, isOutput=False)
    p_w1 = nc.declare_dram_parameter("p_w1", [128, NMF, NKC, 128], dt.bfloat16, isOutput=False)
    p_b1c = nc.declare_dram_parameter("p_b1c", [128, NMF], dt.float32, isOutput=False)
    p_w2 = nc.declare_dram_parameter("p_w2", [128, NKC, NMF, 128], dt.bfloat16, isOutput=False)
    p_b2c = nc.declare_dram_parameter("p_b2c", [128, NKC], dt.float32, isOutput=False)
    p_xts = nc.declare_dram_parameter("p_xts", [128, NKC, TS], dt.bfloat16, isOutput=False)
    p_ind2 = nc.declare_dram_parameter("p_ind2", [2, 128], dt.bfloat16, isOutput=False)
    p_maskd = nc.declare_dram_parameter("p_maskd", [128, 128], dt.bfloat16, isOutput=False)
    p_ident = nc.declare_dram_parameter("p_ident", [128, 128], dt.bfloat16, isOutput=False)
    p_out = nc.declare_dram_parameter("p_out", [C, TS], dt.float32, isOutput=True)

    with tile.TileContext(nc, num_cores=N_CORES) as tc:
        with (
            tc.tile_pool(name="persist", bufs=1) as pp,
            tc.tile_pool(name="dram", bufs=1, space="DRAM") as pdram,
        ):
            # ------------- constants, phase-A-critical DMAs first -------------
            wq = pp.tile([128, NKC, HD2], dt.bfloat16)
            nc.sync.dma_start(wq[:], p_wq[:])
            bqkv = pp.tile([HD2, 3], dt.float32)
            nc.sync.dma_start(bqkv[:], p_bqkv[:])
            ident = pp.tile([128, 128], dt.bfloat16)
            nc.sync.dma_start(ident[:], p_ident[:])
            wk = pp.tile([128, NKC, HD2], dt.bfloat16)
            wv = pp.tile([128, NKC, HD2], dt.bfloat16)
            maskd = pp.tile([128, 128], dt.bfloat16)
            ones128_row = pp.tile([1, 128], dt.bfloat16)
            nc.vector.memset(ones128_row[:], 1.0)
            isc_col = pp.tile([128, 1], dt.bfloat16)   # 1/1024 column for LN2 sums
            nc.vector.memset(isc_col[:], 1.0 / C)
            ind2 = pp.tile([2, 128], dt.bfloat16)      # Z broadcast selector
            b1c = pp.tile([128, NMF], dt.float32)
            b2c = pp.tile([128, NKC], dt.float32)

            # phase C prefetched weights / residual (persist through the run)
            wo_all = pp.tile([128, NKC, NKC, 128], dt.bfloat16)
            w1_all = pp.tile([128, NMF, NKC, 128], dt.bfloat16)
            w2_all = pp.tile([128, NKC, NMF, 128], dt.bfloat16)
            xts = pp.tile([128, NKC, TS], dt.bfloat16)
            # stage-C inputs live in the persistent pool so their DMAs can be
            # emitted inside stage B, right behind each AllToAll
            ctxC = pp.tile([128, NKC, TS], dt.bfloat16)
            zF = pp.tile([2, NKC, TS], dt.bfloat16)

            # collective staging (DRAM)
            cc_in = [pdram.tile([N_CORES, 130, TQ], dt.bfloat16, name=f"ccin{b}")
                     for b in range(B)]
            cc_out = [pdram.tile([N_CORES, 130, TQ], dt.bfloat16, name=f"ccout{b}")
                      for b in range(B)]

            with tc.tile_pool(name="abact", bufs=1) as pab:
                # activation tensors that live through phases A+B only
                qT = pab.tile([128, TT], dt.bfloat16)
                kT = pab.tile([128, TT], dt.bfloat16)
                v = pab.tile([128, NT, 2, 65], dt.bfloat16)
                ctxT = pab.tile([128, TT], dt.bfloat16)

                # ---------------- stage A: QKV ----------------
                with (
                    tc.tile_pool(name="xin", bufs=3) as pxt,
                    tc.tile_pool(name="vtev", bufs=2) as pvte,
                    tc.tile_pool(name="apsum", bufs=3, space="PSUM") as pps_a,
                    tc.tile_pool(name="apsum1", bufs=2, space="PSUM") as pps_a1,
                ):
                    nc.vector.memset(v[:, :, :, 64], 1.0)
                    xnt0 = pxt.tile([128, NKC, 512], dt.bfloat16, tag="xt")
                    nc.sync.dma_start(xnt0[:], p_xn[:, 0, :, :])
                    # remaining params queue behind the first activation chunk
                    nc.sync.dma_start(wk[:], p_wk[:])
                    nc.sync.dma_start(wv[:], p_wv[:])
                    nc.sync.dma_start(maskd[:], p_maskd[:])
                    nc.sync.dma_start(ind2[:], p_ind2[:])
                    nc.sync.dma_start(b1c[:], p_b1c[:])
                    nc.sync.dma_start(b2c[:], p_b2c[:])
                    for ch in range(TT // 512):
                        sl = slice(512 * ch, 512 * (ch + 1))
                        if ch == 0:
                            xnt = xnt0
                        else:
                            xnt = pxt.tile([128, NKC, 512], dt.bfloat16, tag="xt")
                            nc.sync.dma_start(xnt[:], p_xn[:, ch, :, :])
                        vT = pvte.tile([128, 512], dt.bfloat16, tag="vt")
                        for idx, (w, dst) in enumerate(
                                ((wq, qT), (wk, kT), (wv, None))):
                            ps = pps_a.tile([128, 512], dt.float32, tag="qkv")
                            for k in range(NKC):
                                nc.tensor.matmul(ps[:], w[:, k, :], xnt[:, k, :],
                                                 start=(k == 0), stop=(k == NKC - 1))
                            if idx == 0:
                                nc.scalar.activation(dst[:, sl], ps[:], act.Identity,
                                                     bias=bqkv[:, idx:idx + 1])
                            elif idx == 1:
                                nc.vector.tensor_scalar(dst[:, sl], ps[:],
                                                        bqkv[:, idx:idx + 1], None,
                                                        alu.add)
                            else:
                                nc.vector.tensor_scalar(vT[:], ps[:],
                                                        bqkv[:, idx:idx + 1], None,
                                                        alu.add)
                        # v_aug [s, tile, head, 65] via PE transpose of vT
                        for i in range(4):
                            pvt = pps_a1.tile([128, 128], dt.bfloat16, tag="vtp")
                            nc.tensor.transpose(pvt[:], vT[:, 128 * i:128 * (i + 1)],
                                                ident[:])
                            nc.scalar.copy(v[:, 4 * ch + i, :, 0:64],
                                           pvt[:].rearrange("p (h d) -> p h d", h=2))
                        # interleave phase-C prefetch pieces so the DMA queue
                        # stays just ahead of compute without starving the
                        # critical xn chunk loads
                        nc.sync.dma_start(xts[:, ch, :], p_xts[:, ch, :])
                        nc.sync.dma_start(wo_all[:, ch, :, :], p_wo[:, ch, :, :])
                        for mf in (2 * ch, 2 * ch + 1):
                            nc.sync.dma_start(w1_all[:, mf, :, :],
                                              p_w1[:, mf, :, :])

                    # remaining prefetch (finishes early in stage B, ahead of
                    # the first AllToAll's staging DMAs)
                    for mf in range(16, NMF):
                        nc.sync.dma_start(w1_all[:, mf, :, :], p_w1[:, mf, :, :])
                    nc.sync.dma_start(w2_all[:], p_w2[:])

                # ---------------- stage B: attention ----------------
                with (
                    tc.tile_pool(name="exps", bufs=4) as pexp,
                    tc.tile_pool(name="scpsum", bufs=3, space="PSUM") as pps_sc,
                    tc.tile_pool(name="ctxpsum", bufs=1, space="PSUM") as pps_ctx,
                ):
                    for b in range(B):
                        for qt in range(T // 512):
                            G = b * T + 512 * qt
                            gsl = slice(G, G + 512)
                            nj = 4 * qt + 4
                            pcs = pps_ctx.tile([65, 2, 512], dt.float32, tag="ctx")
                            ets = []
                            for j in range(nj):
                                st = b * (T // 128) + j   # global s-tile index
                                sp = pps_sc.tile([128, 2, 512], dt.float32, tag="sc")
                                for h in range(2):
                                    hsl = slice(64 * h, 64 * (h + 1))
                                    nc.tensor.matmul(
                                        sp[:, h, :],
                                        kT[hsl, 128 * st:128 * (st + 1)],
                                        qT[hsl, gsl], start=True, stop=True)
                                et = pexp.tile([128, 2, 512], dt.bfloat16, tag="et")
                                if j >= nj - 4:
                                    off = j - (nj - 4)
                                    if off > 0:
                                        nc.gpsimd.memset(et[:, :, 0:128 * off], 0.0)
                                    for h in range(2):
                                        nc.scalar.activation(
                                            et[:, h, 128 * off:512],
                                            sp[:, h, 128 * off:512],
                                            act.Exp,
                                            scale=1.0 / float(np.sqrt(H)))
                                        nc.gpsimd.tensor_tensor(
                                            et[:, h, 128 * off:128 * (off + 1)],
                                            et[:, h, 128 * off:128 * (off + 1)],
                                            maskd[:], alu.mult)
                                else:
                                    # one single-bank exp per head (faster
                                    # than one activation spanning 2 banks)
                                    for h in range(2):
                                        nc.scalar.activation(
                                            et[:, h, :], sp[:, h, :],
                                            act.Exp,
                                            scale=1.0 / float(np.sqrt(H)))
                                ets.append(et)
                                # software pipeline: AV for tile j-1 after scores j
                                if j > 0:
                                    for h in range(2):
                                        nc.tensor.matmul(
                                            pcs[:, h, :],
                                            v[:, b * (T // 128) + j - 1, h, :],
                                            ets[j - 1][:, h, :],
                                            start=(j - 1 == 0), stop=False)
                            for h in range(2):
                                nc.tensor.matmul(
                                    pcs[:, h, :], v[:, b * (T // 128) + nj - 1, h, :],
                                    ets[nj - 1][:, h, :],
                                    start=(nj == 1), stop=True)
                            # evict raw ctx + Z (normalization deferred to stage C)
                            zc = [pexp.tile([1, 512], dt.bfloat16, tag=f"zc{h}",
                                            name=f"zc{h}")
                                  for h in range(2)]
                            for h in range(2):
                                nc.vector.tensor_copy(ctxT[64 * h:64 * (h + 1), gsl],
                                                      pcs[0:64, h, :])
                                nc.vector.tensor_copy(zc[h][:], pcs[64:65, h, :])
                            # this 512-token chunk feeds dst cores 2qt, 2qt+1
                            for i2, j2 in enumerate((2 * qt, 2 * qt + 1)):
                                tsl = slice(b * T + TQ * j2, b * T + TQ * (j2 + 1))
                                nc.sync.dma_start(cc_in[b][j2, 0:128, :],
                                                  ctxT[:, tsl])
                                for h in range(2):
                                    nc.sync.dma_start(
                                        cc_in[b][j2, 128 + h, :],
                                        zc[h][:, TQ * i2:TQ * (i2 + 1)])
                        nc.gpsimd.collective_compute(
                            "AllToAll", alu.bypass,
                            replica_groups=[list(range(N_CORES))],
                            ins=[cc_in[b].opt()],
                            outs=[cc_out[b].opt()],
                        )
                        # stage-C input DMAs for this half, emitted here so
                        # they run as soon as the AllToAll lands instead of
                        # queueing behind the second batch's staging
                        cslb = slice(TQ * b, TQ * (b + 1))
                        for j2 in range(N_CORES):
                            nc.sync.dma_start(zF[:, j2, cslb],
                                              cc_out[b][j2, 128:130, :])
                            nc.sync.dma_start(ctxC[:, j2, cslb],
                                              cc_out[b][j2, 0:128, :])

            # ---------------- stage C: Wo + LN2 + FFN ----------------
            # processed in two column halves (one per batch element) so half 0
            # starts as soon as AllToAll#0 lands and half 1's weights/stats
            # chain hides under half 0's FFN matmuls
            with (
                tc.tile_pool(name="postsb", bufs=1) as pq,
                tc.tile_pool(name="evict", bufs=3) as pev,
                tc.tile_pool(name="ln2tmp", bufs=1) as pl2,
                tc.tile_pool(name="ffpsum", bufs=3, space="PSUM") as pps_ff,
                tc.tile_pool(name="npsum", bufs=2, space="PSUM") as pps_n,
                tc.tile_pool(name="cpsum", bufs=1, space="PSUM") as pps_c,
            ):
                r2b = pq.tile([128, NKC, TS], dt.bfloat16)
                xn2T = pq.tile([128, NKC, TS], dt.bfloat16)
                hT = pq.tile([128, NMF, TS], dt.bfloat16)

                for half in range(B):
                    csl = slice(TQ * half, TQ * (half + 1))
                    # normalize ctx by 1/Z in place: broadcast Z via K=2
                    # matmul, reciprocal on the broadcast tile, multiply
                    for k in range(NKC):
                        pz = pps_n.tile([128, TQ], dt.float32, tag="nz")
                        nc.tensor.matmul(pz[:], ind2[:], zF[:, k, csl],
                                         start=True, stop=True)
                        zinvb = pev.tile([128, TQ], dt.float32, tag="zi")
                        nc.vector.reciprocal_approx_fast(zinvb[:], pz[:])
                        nc.vector.tensor_tensor(ctxC[:, k, csl], ctxC[:, k, csl],
                                                zinvb[:], alu.mult)

                    # Wo + residual (residual kept in bf16)
                    for mc in range(NKC):
                        ps = pps_ff.tile([128, TQ], dt.float32, tag="ff")
                        for k in range(NKC):
                            nc.tensor.matmul(ps[:], wo_all[:, mc, k, :],
                                             ctxC[:, k, csl],
                                             start=(k == 0), stop=(k == NKC - 1))
                        nc.vector.tensor_tensor(r2b[:, mc, csl], ps[:],
                                                xts[:, mc, csl], alu.add)

                    # LN2 partition sums (mean, mean of square)
                    ps1 = pps_c.tile([1, TQ], dt.float32, tag="s1")
                    ps2 = pps_c.tile([1, TQ], dt.float32, tag="s2")
                    for mc in range(NKC):
                        sqt = pev.tile([128, TQ], dt.bfloat16, tag="sq")
                        nc.gpsimd.tensor_tensor(sqt[:], r2b[:, mc, csl],
                                                r2b[:, mc, csl], alu.mult)
                        nc.tensor.matmul(ps1[:], isc_col[:], r2b[:, mc, csl],
                                         start=(mc == 0), stop=(mc == NKC - 1))
                        nc.tensor.matmul(ps2[:], isc_col[:], sqt[:],
                                         start=(mc == 0), stop=(mc == NKC - 1))
                    muf = pl2.tile([1, TQ], dt.float32, tag="muf")
                    nc.vector.tensor_copy(muf[:], ps1[:])
                    varf = pl2.tile([1, TQ], dt.float32, tag="varf")
                    nc.vector.tensor_tensor(varf[:], muf[:], muf[:], alu.mult)
                    nc.vector.tensor_tensor(varf[:], ps2[:], varf[:], alu.subtract)
                    sdr = pl2.tile([1, TQ], dt.float32, tag="sdr")
                    nc.scalar.activation(sdr[:], varf[:], act.Sqrt,
                                         scale=float(C) / (C - 1))
                    mu2row = pl2.tile([1, TQ], dt.bfloat16, tag="mu2")
                    nc.vector.tensor_copy(mu2row[:], muf[:])
                    sd2row = pl2.tile([1, TQ], dt.bfloat16, tag="sd2")
                    nc.vector.tensor_copy(sd2row[:], sdr[:])
                    pmb = pps_c.tile([128, TQ], dt.float32, tag="bcast")
                    nc.tensor.matmul(pmb[:], ones128_row[:], mu2row[:],
                                     start=True, stop=True)
                    m2b = pl2.tile([128, TQ], dt.bfloat16, tag="m2b")
                    nc.scalar.copy(m2b[:], pmb[:])
                    pib = pps_c.tile([128, TQ], dt.float32, tag="bcast")
                    nc.tensor.matmul(pib[:], ones128_row[:], sd2row[:],
                                     start=True, stop=True)
                    i2b = pl2.tile([128, TQ], dt.float32, tag="i2b")
                    nc.vector.reciprocal_approx_fast(i2b[:], pib[:])

                    for mc in range(NKC):
                        tmp = pev.tile([128, TQ], dt.bfloat16, tag="xtmp")
                        nc.gpsimd.tensor_tensor(tmp[:], r2b[:, mc, csl], m2b[:],
                                                alu.subtract)
                        nc.vector.tensor_tensor(xn2T[:, mc, csl], tmp[:], i2b[:],
                                                alu.mult)

                    # ---- FFN ----
                    for mf in range(NMF):
                        ps = pps_ff.tile([128, TQ], dt.float32, tag="ff")
                        for k in range(NKC):
                            nc.tensor.matmul(ps[:], w1_all[:, mf, k, :],
                                             xn2T[:, k, csl],
                                             start=(k == 0), stop=(k == NKC - 1))
                        nc.scalar.activation(hT[:, mf, csl], ps[:], act.Relu,
                                             bias=b1c[:, mf:mf + 1])

                    for mc in range(NKC):
                        ps = pps_ff.tile([128, TQ], dt.float32, tag="ff")
                        for k in range(NMF):
                            nc.tensor.matmul(ps[:], w2_all[:, mc, k, :],
                                             hT[:, k, csl],
                                             start=(k == 0), stop=(k == NMF - 1))
                        ot = pev.tile([128, TQ], dt.float32, tag="ot")
                        nc.vector.scalar_tensor_tensor(ot[:], ps[:],
                                                       b2c[:, mc:mc + 1],
                                                       r2b[:, mc, csl],
                                                       alu.add, alu.add)
                        nc.sync.dma_start(p_out[128 * mc:128 * (mc + 1), csl],
                                          ot[:])

    nc.compile()
    return nc


def _host_prep(inputs):
    """Fold LN affines into weights, apply LN1 on host, build per-core maps.

    All device-visible arrays are laid out partition-major ([128, ...]) so
    DMAs move long contiguous lines per partition.
    """
    x = np.asarray(inputs["x"], np.float32)
    Wq = np.asarray(inputs["Wq"], np.float32)
    Wk = np.asarray(inputs["Wk"], np.float32)
    Wv = np.asarray(inputs["Wv"], np.float32)
    Wo = np.asarray(inputs["Wo"], np.float32)
    bo = np.asarray(inputs["bo"], np.float32)
    W1 = np.asarray(inputs["W1"], np.float32)
    b1 = np.asarray(inputs["b1"], np.float32)
    W2 = np.asarray(inputs["W2"], np.float32)
    b2 = np.asarray(inputs["b2"], np.float32)
    g1 = np.asarray(inputs["g1"], np.float32)
    be1 = np.asarray(inputs["be1"], np.float32)
    g2 = np.asarray(inputs["g2"], np.float32)
    be2 = np.asarray(inputs["be2"], np.float32)

    xf = x.reshape(TT, C)                      # both batches stacked
    # LN1 on host (elementwise prep; torch: unbiased std, eps added to std)
    mu = xf.mean(axis=1, keepdims=True)
    sd = np.sqrt(xf.var(axis=1, ddof=1, keepdims=True)) + EPS
    xn = (xf - mu) / sd                        # gamma folded into Wq/Wk/Wv
    # [C, TT] -> partition-major [128, n_chunk, NKC, 512]
    xnP = np.ascontiguousarray(
        xn.T.reshape(NKC, 128, TT // 512, 512).transpose(1, 2, 0, 3))

    def fold_qkv(W):
        Weff = g1[:, None] * W                  # [NH, C, H] with g1 on C
        Weff = np.ascontiguousarray(np.transpose(Weff, (1, 0, 2)))  # [C, NH, H]
        bias = np.einsum("c,hck->hk", be1, W)   # [NH, H]
        return Weff, bias

    Wq_e, bq = fold_qkv(Wq)
    Wk_e, bk = fold_qkv(Wk)
    Wv_e, bv = fold_qkv(Wv)

    woT = np.ascontiguousarray(Wo.T)            # [NH*H, C]
    w1T = np.ascontiguousarray(g2[:, None] * W1.T)   # [C, FF]
    b1_eff = b1 + be2 @ W1.T                         # [FF]
    w2T = np.ascontiguousarray(W2.T)            # [FF, C]

    # partition-major blocked weights
    # wo: [C, C] -> [p, mc, k, 128] with row k*128+p of block mc
    woP = np.ascontiguousarray(
        woT.reshape(NKC, 128, NKC, 128).transpose(1, 2, 0, 3))
    w1P = np.ascontiguousarray(
        w1T.reshape(NKC, 128, NMF, 128).transpose(1, 2, 0, 3))
    w2P = np.ascontiguousarray(
        w2T.reshape(NMF, 128, NKC, 128).transpose(1, 2, 0, 3))

    tq = np.arange(128)[None, :]
    s = np.arange(128)[:, None]
    maskd = (s <= tq).astype(BF16)

    shared = {
        "p_xn": xnP.astype(BF16),
        "p_wo": woP.astype(BF16),
        "p_w1": w1P.astype(BF16),
        "p_b1c": np.ascontiguousarray(
            b1_eff.reshape(NMF, 128).T).astype(np.float32),
        "p_w2": w2P.astype(BF16),
        "p_b2c": np.ascontiguousarray(
            b2.reshape(NKC, 128).T).astype(np.float32),
        "p_ind2": np.repeat(np.eye(2, dtype=np.float32), 64, axis=1).astype(BF16),
        "p_maskd": maskd,
        "p_ident": np.eye(128, dtype=np.float32).astype(BF16),
    }

    in_maps = []
    for r in range(N_CORES):
        h0 = HPC * r
        hs = slice(h0, h0 + HPC)
        m = dict(shared)
        for nm, We in (("p_wq", Wq_e), ("p_wk", Wk_e), ("p_wv", Wv_e)):
            wr = We[:, hs, :].reshape(C, HD2)        # [C, 128]
            m[nm] = np.ascontiguousarray(
                wr.reshape(NKC, 128, HD2).transpose(1, 0, 2)).astype(BF16)
        m["p_bqkv"] = np.ascontiguousarray(
            np.stack([bq[hs].reshape(HD2), bk[hs].reshape(HD2),
                      bv[hs].reshape(HD2)], axis=1)).astype(np.float32)
        # residual stream for this core's tokens: 256 from each batch,
        # with the Wo bias folded in; partition-major [128, NKC, TS]
        xts = np.concatenate(
            [x[b, TQ * r:TQ * (r + 1), :].T for b in range(B)], axis=1)
        xts = xts + bo[:, None]                      # [C, TS]
        m["p_xts"] = np.ascontiguousarray(
            xts.reshape(NKC, 128, TS).transpose(1, 0, 2)).astype(BF16)
        in_maps.append(m)
    return in_maps


def kernel(**inputs) -> np.ndarray:
    from concourse.bass_utils import run_bass_kernel_spmd

    if "nc" not in _BUILT:
        _BUILT["nc"] = _build()
    nc = _BUILT["nc"]

    in_maps = _host_prep(inputs)
    res = run_bass_kernel_spmd(nc, in_maps, core_ids=list(range(N_CORES)))

    out = np.empty((B, T, C), np.float32)
    for r in range(N_CORES):
        po = res.results[r]["p_out"]
        for b in range(B):
            out[b, TQ * r:TQ * (r + 1), :] = po[:, TQ * b:TQ * (b + 1)].T
    return out
